# revision 16
# baseline (speedup 1.0000x reference)
"""Two-layer SAGEConv + linear head on Trainium2 (8 NeuronCores, SPMD).

Strategy: shard destination nodes across 8 cores (6250/core, 49 tiles of 128).
Edges are bucketed host-side by (core, dst_tile, table_half) and padded to
128-slot chunks.  Per chunk the core gathers x[src] rows from a DRAM table
(dma_gather, bf16 256B rows with a ones column for degree counting), builds a
one-hot dst matrix with a single tensor_scalar(is_equal) against an iota
constant, and accumulates O^T @ G into PSUM.  Mean + the two dense linear
terms + relu run per tile on TensorE/DVE.  Between layers one AllGather
shares h @ W2_l^T (32-dim) across cores; layer 2 repeats the same machinery
reading from that table.  Output is [1, 6272] f32 per core, assembled on host.
"""
import sys, os

sys.path.insert(0, "/opt/trn_rl_repo")

import numpy as np
import ml_dtypes

import concourse.bass as bass
import concourse.bacc as bacc
import concourse.mybir as mybir
import concourse.tile as tile
from concourse.bass_utils import run_bass_kernel_spmd
from concourse.library_config import mlp

BF16 = mybir.dt.bfloat16
F32 = mybir.dt.float32
I16 = mybir.dt.int16

_LAST_EXEC_NS = None


def _prep_edges(edge_index, cfg):
    """Bucket/pad edges host-side. Returns per-core idx/dstloc arrays + chunk table."""
    NPC, NLOC, NTIL, NC, HALF = (cfg["NPC"], cfg["NLOC"], cfg["NTIL"],
                                 cfg["NC"], cfg["HALF"])
    src = np.asarray(edge_index[0], dtype=np.int64)
    dst = np.asarray(edge_index[1], dtype=np.int64)
    pid_src = (src // NPC) * NLOC + (src % NPC)
    core = dst // NPC
    tl = (dst % NPC) // 128
    dl = (dst % NPC) % 128
    half = (pid_src >= HALF).astype(np.int64)
    idx16 = (pid_src - half * HALF).astype(np.int16)

    key = ((core * NTIL) + tl) * 2 + half
    order = np.argsort(key, kind="stable")
    key_s = key[order]
    idx_s = idx16[order]
    dl_s = dl[order].astype(np.int16)

    ngroups = NC * NTIL * 2
    bounds = np.searchsorted(key_s, np.arange(ngroups + 1))
    cnt = bounds[1:] - bounds[:-1]              # [NC*NTIL*2]
    cnt = cnt.reshape(NC, NTIL, 2)
    nch = np.maximum(np.ceil(cnt / 128).astype(np.int64).max(axis=0), 0)  # [NTIL,2]

    tot_ch = int(nch.sum())
    idx_arr = np.zeros((NC, 128, tot_ch * 8), dtype=np.int16)
    dst_arr = np.full((NC, 128, tot_ch), -1.0, dtype=np.float32)
    ch_off = {}
    off = 0
    for h in range(2):
        for t in range(NTIL):
            ch_off[(t, h)] = off
            off += int(nch[t, h])

    for c in range(NC):
        for t in range(NTIL):
            for h in range(2):
                n = int(cnt[c, t, h])
                nchunks = int(nch[t, h])
                if nchunks == 0:
                    continue
                g0 = bounds[((c * NTIL) + t) * 2 + h]
                pad = nchunks * 128
                iv = np.zeros(pad, dtype=np.int16)
                dv = np.full(pad, -1.0, dtype=np.float32)
                iv[:n] = idx_s[g0:g0 + n]
                dv[:n] = dl_s[g0:g0 + n].astype(np.float32)
                o = ch_off[(t, h)]
                # dstloc: slot p of chunk k -> [p, o+k]
                dst_arr[c, :, o:o + nchunks] = dv.reshape(nchunks, 128).T
                # idx: i -> [i%16, o*8 + i//16], replicated over 8 groups
                iw = iv.reshape(nchunks * 8, 16).T  # [16, nchunks*8]
                idx_arr[c, :16, o * 8:(o + nchunks) * 8] = iw
        idx_arr[c] = np.tile(idx_arr[c, :16], (8, 1))
    return idx_arr, dst_arr, nch, ch_off, tot_ch


def _build(cfg, nch, ch_off, tot_ch):
    NPC, NLOC, NTIL, NC, HALF, NT = (cfg["NPC"], cfg["NLOC"], cfg["NTIL"],
                                     cfg["NC"], cfg["HALF"], cfg["NTAB"])
    nc = bacc.Bacc("TRN2", target_bir_lowering=False, debug=False, num_swdge_queues=4)
    dram = lambda n, s, d: nc.dram_tensor(n, s, d, kind="ExternalInput")
    xtabA = dram("xtabA", [HALF, 128], BF16)
    xtabB = dram("xtabB", [NT - HALF, 128], BF16)
    idx_d = dram("idx", [128, tot_ch * 8], I16)
    dstl_d = dram("dstl", [128, tot_ch], F32)
    xT_d = dram("xT", [64, NLOC], BF16)
    w1l_d = dram("W1lT", [64, 64], BF16)
    w1r_d = dram("W1rT", [64, 64], BF16)
    w2l_d = dram("W2lT", [64, 32], BF16)
    w2r_d = dram("W2rT", [64, 32], BF16)
    wln_d = dram("WlinT", [32, 1], BF16)
    b1_d = dram("b1", [128, 64], F32)
    b2_d = dram("b2", [128, 32], F32)
    bl_d = dram("blin", [1, 1], F32)
    c_d = dram("Ciota", [128, 128], F32)
    id_d = dram("Ident", [128, 128], BF16)
    out_d = nc.dram_tensor("out", [1, NLOC], F32, kind="ExternalOutput")

    AG = cfg["NC"] > 1
    with tile.TileContext(nc) as tc:
        with (
            tc.tile_pool(name="const", bufs=1) as cpool,
            tc.tile_pool(name="sb", bufs=6) as sb,
            tc.tile_pool(name="gt", bufs=16) as gp,
            tc.tile_pool(name="pa", bufs=3, space="PSUM") as pa,
            tc.tile_pool(name="pb", bufs=4, space="PSUM") as pb,
            tc.tile_pool(name="dram", bufs=1, space="DRAM") as dp,
        ):
            nc.gpsimd.load_library(mlp)
            # constants / caches -- idx/dstl first (gathers gate on them)
            idx_sb = cpool.tile([128, tot_ch * 8], I16)
            _qs = tot_ch * 8 // 4
            for _i in range(4):
                _lo = _i * _qs
                _hi = (tot_ch * 8) if _i == 3 else (_lo + _qs)
                nc.sync.dma_start(out=idx_sb[:, _lo:_hi], in_=idx_d[:, _lo:_hi])
            dstl_sb = cpool.tile([128, tot_ch], F32)
            nc.scalar.dma_start(out=dstl_sb[:], in_=dstl_d[:, :])
            xT_sb = cpool.tile_from(xT_d[:, :])
            w1l = cpool.tile_from(w1l_d[:, :])
            w1r = cpool.tile_from(w1r_d[:, :])
            w2l = cpool.tile_from(w2l_d[:, :])
            w2r = cpool.tile_from(w2r_d[:, :])
            wln = cpool.tile_from(wln_d[:, :])
            b1 = cpool.tile_from(b1_d[:, :])
            b2 = cpool.tile_from(b2_d[:, :])
            bl = cpool.tile_from(bl_d[:, :])
            ci = cpool.tile_from(c_d[:, :])
            ident = cpool.tile_from(id_d[:, :])
            recip = cpool.tile([128, NTIL], F32)
            neg1 = cpool.tile([128, 1], F32)
            nc.vector.memset(neg1[:], -1.0)
            pos1 = cpool.tile([128, 1], F32)
            nc.vector.memset(pos1[:], 1.0)
            hT_cache = cpool.tile([64, NTIL * 128], BF16)
            out_sb = cpool.tile([1, NLOC], F32)

            hw2l_loc = dp.tile([NLOC, 32], BF16)
            if AG:
                hw2l_all = dp.tile([NT, 32], BF16)
            else:
                hw2l_all = hw2l_loc
            l2tA = dp.tile([HALF, 128], BF16)
            l2tB = dp.tile([NT - HALF, 128], BF16)

            qctr = [0]
            TOTH = [int(sum(nch[t2, 0] for t2 in range(NTIL))),
                    int(sum(nch[t2, 1] for t2 in range(NTIL)))]
            stream = {0: [], 1: []}

            def ensure_call(h, ci_, tabA, tabB):
                lst = stream[h]
                base = 0 if h == 0 else TOTH[0]
                tabs = tabA if h == 0 else tabB
                while len(lst) <= ci_:
                    j = len(lst) * 8
                    k = min(8, TOTH[h] - j)
                    og = base + j
                    g = gp.tile([128, k, 128], BF16, tag="G")
                    nc.gpsimd.dma_gather(
                        g[:], tabs[:], idx_sb[:, og * 8:(og + k) * 8],
                        k * 128, k * 128, 128, queue_num=qctr[0] % 4)
                    qctr[0] += 1
                    ob = sb.tile([128, k, 128], BF16, tag="O")
                    nc.vector.tensor_tensor(
                        out=ob[:],
                        in0=ci[:, None, :].to_broadcast([128, k, 128]),
                        in1=dstl_sb[:, og:og + k, None].to_broadcast([128, k, 128]),
                        op=mybir.AluOpType.is_equal)
                    lst.append((g, ob, k))
                return lst[ci_]

            def agg_layer(t, tabA, tabB, width, ps_pool):
                """One-hot aggregation for tile t -> psum [128, width]."""
                ps = ps_pool.tile([128, width], F32, tag="agg")
                total = int(nch[t, 0] + nch[t, 1])
                done = 0
                for h in (0, 1):
                    off_t = ch_off[(t, h)] - (0 if h == 0 else TOTH[0])
                    for l in range(int(nch[t, h])):
                        sc = off_t + l
                        g, ob, k = ensure_call(h, sc // 8, tabA, tabB)
                        c = sc % 8
                        nc.tensor.matmul(
                            out=ps[:], lhsT=ob[:, c, :], rhs=g[:, c, :width],
                            start=(done == 0), stop=(done == total - 1))
                        done += 1
                return ps

            # ---------------- Layer 1 ----------------
            for t in range(NTIL):
                ps = agg_layer(t, xtabA, xtabB, 65, pa)
                tmp1 = sb.tile([128, 1], F32, tag="c1")
                nc.vector.tensor_scalar_max(tmp1[:], ps[:, 64:65], 1.0)
                nc.vector.reciprocal(recip[:, t:t + 1], tmp1[:])
                aggs = sb.tile([128, 64], BF16, tag="aggs")
                nc.vector.tensor_scalar(
                    out=aggs[:], in0=ps[:, :64], scalar1=recip[:, t:t + 1],
                    scalar2=None, op0=mybir.AluOpType.mult)
                pT = pb.tile([64, 128], BF16, tag="pb")
                nc.tensor.transpose(out=pT[:], in_=aggs[:], identity=ident[:])
                aggT = sb.tile([64, 128], BF16, tag="aggT")
                nc.any.tensor_copy(out=aggT[:], in_=pT[:])
                pH = pb.tile([128, 64], F32, tag="pb")
                nc.tensor.matmul(out=pH[:], lhsT=aggT[:], rhs=w1l[:],
                                 start=True, stop=False)
                nc.tensor.matmul(out=pH[:], lhsT=xT_sb[:, t * 128:(t + 1) * 128],
                                 rhs=w1r[:], start=False, stop=True)
                hf = sb.tile([128, 64], F32, tag="hf")
                nc.vector.tensor_tensor(out=hf[:], in0=pH[:], in1=b1[:],
                                        op=mybir.AluOpType.add)
                hb = sb.tile([128, 64], BF16, tag="hb")
                nc.scalar.activation(hb[:], hf[:], mybir.ActivationFunctionType.Relu)
                pT2 = pb.tile([64, 128], BF16, tag="pb")
                nc.tensor.transpose(out=pT2[:], in_=hb[:], identity=ident[:])
                hTs = hT_cache[:, t * 128:(t + 1) * 128]
                nc.any.tensor_copy(out=hTs, in_=pT2[:])
                pW = pb.tile([128, 32], F32, tag="pb")
                nc.tensor.matmul(out=pW[:], lhsT=hTs, rhs=w2l[:],
                                 start=True, stop=True)
                wsb = sb.tile([128, 32], BF16, tag="wsb")
                nc.any.tensor_copy(out=wsb[:], in_=pW[:])
                nc.sync.dma_start(out=hw2l_loc[t * 128:(t + 1) * 128, :], in_=wsb[:])

            # ---------------- AllGather + L2 table expand ----------------
            if AG:
                nc.gpsimd.collective_compute(
                    "AllGather", mybir.AluOpType.bypass,
                    replica_groups=[list(range(NC))],
                    ins=[hw2l_loc.opt()], outs=[hw2l_all.opt()])
            HH = HALF // 2
            nc.sync.dma_start(out=l2tA[0:HH, :32], in_=hw2l_all[0:HH, :])
            nc.scalar.dma_start(out=l2tA[HH:HALF, :32], in_=hw2l_all[HH:HALF, :])
            nc.sync.dma_start(out=l2tB[0:HH, :32], in_=hw2l_all[HALF:HALF + HH, :])
            nc.scalar.dma_start(out=l2tB[HH:NT - HALF, :32],
                                in_=hw2l_all[HALF + HH:NT, :])

            # ---------------- Layer 2 ----------------
            stream[0], stream[1] = [], []
            for t in range(NTIL):
                ps = agg_layer(t, l2tA, l2tB, 32, pa)
                a2 = sb.tile([128, 32], F32, tag="a2")
                nc.vector.tensor_scalar(
                    out=a2[:], in0=ps[:], scalar1=recip[:, t:t + 1],
                    scalar2=None, op0=mybir.AluOpType.mult)
                pH2 = pb.tile([128, 32], F32, tag="pb")
                nc.tensor.matmul(out=pH2[:], lhsT=hT_cache[:, t * 128:(t + 1) * 128],
                                 rhs=w2r[:], start=True, stop=True)
                h2f = sb.tile([128, 32], F32, tag="h2f")
                nc.vector.tensor_tensor(out=h2f[:], in0=pH2[:], in1=a2[:],
                                        op=mybir.AluOpType.add)
                nc.vector.tensor_tensor(out=h2f[:], in0=h2f[:], in1=b2[:],
                                        op=mybir.AluOpType.add)
                h2b = sb.tile([128, 32], BF16, tag="h2b")
                nc.scalar.activation(h2b[:], h2f[:], mybir.ActivationFunctionType.Relu)
                pT3 = pb.tile([32, 128], BF16, tag="pb")
                nc.tensor.transpose(out=pT3[:], in_=h2b[:], identity=ident[:])
                h2T = sb.tile([32, 128], BF16, tag="h2T")
                nc.any.tensor_copy(out=h2T[:], in_=pT3[:])
                pO = pb.tile([1, 128], F32, tag="pb")
                nc.tensor.matmul(out=pO[:], lhsT=wln[:], rhs=h2T[:],
                                 start=True, stop=True)
                nc.vector.tensor_scalar(
                    out=out_sb[0:1, t * 128:(t + 1) * 128], in0=pO[:],
                    scalar1=bl[0:1, 0:1], scalar2=None, op0=mybir.AluOpType.add)
                if t % 12 == 11 or t == NTIL - 1:
                    lo = (t // 12) * 12 * 128
                    nc.sync.dma_start(out=out_d[:, lo:(t + 1) * 128],
                                      in_=out_sb[0:1, lo:(t + 1) * 128])
    nc.compile()
    return nc


def _run(x, edge_index, W1_l, b1_l, W1_r, W2_l, b2_l, W2_r, W_lin, b_lin, cfg,
         trace=False):
    global _LAST_EXEC_NS
    N, NC, NPC, NLOC, NTIL, HALF, NT = (cfg["N"], cfg["NC"], cfg["NPC"],
                                        cfg["NLOC"], cfg["NTIL"], cfg["HALF"],
                                        cfg["NTAB"])
    x = np.asarray(x, dtype=np.float32)
    idx_arr, dst_arr, nch, ch_off, tot_ch = _prep_edges(edge_index, cfg)
    nc = _build(cfg, nch, ch_off, tot_ch)

    bf = ml_dtypes.bfloat16
    # gather table: rows [x | 1 | 0...] in permuted (padded) node order
    xtab = np.zeros((NT, 128), dtype=bf)
    xp = np.zeros((NT, 64), dtype=np.float32)
    for c in range(NC):
        xp[c * NLOC:c * NLOC + NPC] = x[c * NPC:(c + 1) * NPC]
    xtab[:, :64] = xp.astype(bf)
    xtab[:, 64] = bf(1.0)
    xtabA, xtabB = xtab[:HALF], xtab[HALF:]

    b1_bc = np.tile(np.asarray(b1_l, np.float32)[None, :], (128, 1))
    b2_bc = np.tile(np.asarray(b2_l, np.float32)[None, :], (128, 1))
    bl_bc = np.asarray(b_lin, np.float32).reshape(1, 1)
    ciota = np.tile(np.arange(128, dtype=np.float32)[None, :], (128, 1))
    ident = np.eye(128, dtype=np.float32).astype(bf)

    common = {
        "xtabA": xtabA, "xtabB": xtabB,
        "W1lT": np.asarray(W1_l, np.float32).T.copy().astype(bf),
        "W1rT": np.asarray(W1_r, np.float32).T.copy().astype(bf),
        "W2lT": np.asarray(W2_l, np.float32).T.copy().astype(bf),
        "W2rT": np.asarray(W2_r, np.float32).T.copy().astype(bf),
        "WlinT": np.asarray(W_lin, np.float32).T.copy().astype(bf),
        "b1": b1_bc, "b2": b2_bc, "blin": bl_bc,
        "Ciota": ciota, "Ident": ident,
    }
    in_maps = []
    for c in range(NC):
        xl = np.zeros((NLOC, 64), dtype=np.float32)
        xl[:NPC] = x[c * NPC:(c + 1) * NPC]
        m = dict(common)
        m["idx"] = idx_arr[c]
        m["dstl"] = np.asarray(dst_arr[c])
        m["xT"] = xl.T.copy().astype(bf)
        in_maps.append(m)

    res = run_bass_kernel_spmd(nc, in_maps, core_ids=list(range(NC)), trace=trace)
    _LAST_EXEC_NS = res.exec_time_ns
    out = np.zeros((N, 1), dtype=np.float32)
    for c in range(NC):
        out[c * NPC:(c + 1) * NPC, 0] = res.results[c]["out"][0, :NPC]
    return out


def _mkcfg(N, NC):
    NPC = N // NC
    NTIL = (NPC + 127) // 128
    NLOC = NTIL * 128
    NT = NC * NLOC
    return {"N": N, "NC": NC, "NPC": NPC, "NTIL": NTIL, "NLOC": NLOC,
            "NTAB": NT, "HALF": NT // 2}


def kernel(x, edge_index, W1_l, b1_l, W1_r, W2_l, b2_l, W2_r, W_lin, b_lin):
    cfg = _mkcfg(50000, 8)
    return _run(x, edge_index, W1_l, b1_l, W1_r, W2_l, b2_l, W2_r, W_lin, b_lin,
                cfg, trace=os.environ.get("BASS_GNN_TRACE", "0") == "1")


# revision 17
# speedup vs baseline: 1.0100x; 1.0100x over previous
"""Two-layer SAGEConv + linear head on Trainium2 (8 NeuronCores, SPMD).

Strategy: shard destination nodes across 8 cores (6250/core, 49 tiles of 128).
Edges are bucketed host-side by (core, dst_tile, table_half) and padded to
128-slot chunks.  Per chunk the core gathers x[src] rows from a DRAM table
(dma_gather, bf16 256B rows with a ones column for degree counting), builds a
one-hot dst matrix with a single tensor_scalar(is_equal) against an iota
constant, and accumulates O^T @ G into PSUM.  Mean + the two dense linear
terms + relu run per tile on TensorE/DVE.  Between layers one AllGather
shares h @ W2_l^T (32-dim) across cores; layer 2 repeats the same machinery
reading from that table.  Output is [1, 6272] f32 per core, assembled on host.
"""
import sys, os

sys.path.insert(0, "/opt/trn_rl_repo")

import numpy as np
import ml_dtypes

import concourse.bass as bass
import concourse.bacc as bacc
import concourse.mybir as mybir
import concourse.tile as tile
from concourse.bass_utils import run_bass_kernel_spmd
from concourse.library_config import mlp

BF16 = mybir.dt.bfloat16
F32 = mybir.dt.float32
I16 = mybir.dt.int16

_LAST_EXEC_NS = None


def _prep_edges(edge_index, cfg):
    """Bucket/pad edges host-side. Returns per-core idx/dstloc arrays + chunk table."""
    NPC, NLOC, NTIL, NC, HALF = (cfg["NPC"], cfg["NLOC"], cfg["NTIL"],
                                 cfg["NC"], cfg["HALF"])
    src = np.asarray(edge_index[0], dtype=np.int64)
    dst = np.asarray(edge_index[1], dtype=np.int64)
    pid_src = (src // NPC) * NLOC + (src % NPC)
    core = dst // NPC
    tl = (dst % NPC) // 128
    dl = (dst % NPC) % 128
    half = (pid_src >= HALF).astype(np.int64)
    idx16 = (pid_src - half * HALF).astype(np.int16)

    key = ((core * NTIL) + tl) * 2 + half
    order = np.argsort(key, kind="stable")
    key_s = key[order]
    idx_s = idx16[order]
    dl_s = dl[order].astype(np.int16)

    ngroups = NC * NTIL * 2
    bounds = np.searchsorted(key_s, np.arange(ngroups + 1))
    cnt = bounds[1:] - bounds[:-1]              # [NC*NTIL*2]
    cnt = cnt.reshape(NC, NTIL, 2)
    nch = np.maximum(np.ceil(cnt / 128).astype(np.int64).max(axis=0), 0)  # [NTIL,2]

    tot_ch = int(nch.sum())
    idx_arr = np.zeros((NC, 128, tot_ch * 8), dtype=np.int16)
    dst_arr = np.full((NC, 128, tot_ch), -1.0, dtype=np.float32)
    ch_off = {}
    off = 0
    for h in range(2):
        for t in range(NTIL):
            ch_off[(t, h)] = off
            off += int(nch[t, h])

    for c in range(NC):
        for t in range(NTIL):
            for h in range(2):
                n = int(cnt[c, t, h])
                nchunks = int(nch[t, h])
                if nchunks == 0:
                    continue
                g0 = bounds[((c * NTIL) + t) * 2 + h]
                pad = nchunks * 128
                iv = np.zeros(pad, dtype=np.int16)
                dv = np.full(pad, -1.0, dtype=np.float32)
                iv[:n] = idx_s[g0:g0 + n]
                dv[:n] = dl_s[g0:g0 + n].astype(np.float32)
                o = ch_off[(t, h)]
                # dstloc: slot p of chunk k -> [p, o+k]
                dst_arr[c, :, o:o + nchunks] = dv.reshape(nchunks, 128).T
                # idx: i -> [i%16, o*8 + i//16], replicated over 8 groups
                iw = iv.reshape(nchunks * 8, 16).T  # [16, nchunks*8]
                idx_arr[c, :16, o * 8:(o + nchunks) * 8] = iw
        idx_arr[c] = np.tile(idx_arr[c, :16], (8, 1))
    return idx_arr, dst_arr, nch, ch_off, tot_ch


def _build(cfg, nch, ch_off, tot_ch):
    NPC, NLOC, NTIL, NC, HALF, NT = (cfg["NPC"], cfg["NLOC"], cfg["NTIL"],
                                     cfg["NC"], cfg["HALF"], cfg["NTAB"])
    nc = bacc.Bacc("TRN2", target_bir_lowering=False, debug=False, num_swdge_queues=4)
    dram = lambda n, s, d: nc.dram_tensor(n, s, d, kind="ExternalInput")
    xtabA = dram("xtabA", [HALF, 128], BF16)
    xtabB = dram("xtabB", [NT - HALF, 128], BF16)
    idx_d = dram("idx", [128, tot_ch * 8], I16)
    dstl_d = dram("dstl", [128, tot_ch], F32)
    xT_d = dram("xT", [64, NLOC], BF16)
    w1l_d = dram("W1lT", [64, 64], BF16)
    w1r_d = dram("W1rT", [64, 64], BF16)
    w2l_d = dram("W2lT", [64, 32], BF16)
    w2r_d = dram("W2rT", [64, 32], BF16)
    wln_d = dram("WlinT", [32, 1], BF16)
    b1_d = dram("b1", [128, 64], F32)
    b2_d = dram("b2", [128, 32], F32)
    bl_d = dram("blin", [1, 1], F32)
    c_d = dram("Ciota", [128, 128], F32)
    id_d = dram("Ident", [128, 128], BF16)
    out_d = nc.dram_tensor("out", [1, NLOC], F32, kind="ExternalOutput")

    AG = cfg["NC"] > 1
    with tile.TileContext(nc) as tc:
        with (
            tc.tile_pool(name="const", bufs=1) as cpool,
            tc.tile_pool(name="sb", bufs=6) as sb,
            tc.tile_pool(name="gt", bufs=12) as gp,
            tc.tile_pool(name="pa", bufs=2, space="PSUM") as pa,
            tc.tile_pool(name="pb", bufs=4, space="PSUM") as pb,
            tc.tile_pool(name="dram", bufs=1, space="DRAM") as dp,
        ):
            nc.gpsimd.load_library(mlp)
            # constants / caches -- idx/dstl first (gathers gate on them)
            idx_sb = cpool.tile([128, tot_ch * 8], I16)
            _qs = tot_ch * 8 // 4
            for _i in range(4):
                _lo = _i * _qs
                _hi = (tot_ch * 8) if _i == 3 else (_lo + _qs)
                nc.sync.dma_start(out=idx_sb[:, _lo:_hi], in_=idx_d[:, _lo:_hi])
            dstl_sb = cpool.tile([128, tot_ch], F32)
            nc.scalar.dma_start(out=dstl_sb[:], in_=dstl_d[:, :])
            xT_sb = cpool.tile_from(xT_d[:, :])
            w1l = cpool.tile_from(w1l_d[:, :])
            w1r = cpool.tile_from(w1r_d[:, :])
            w2l = cpool.tile_from(w2l_d[:, :])
            w2r = cpool.tile_from(w2r_d[:, :])
            wln = cpool.tile_from(wln_d[:, :])
            b1 = cpool.tile_from(b1_d[:, :])
            b2 = cpool.tile_from(b2_d[:, :])
            bl = cpool.tile_from(bl_d[:, :])
            ci = cpool.tile_from(c_d[:, :])
            ident = cpool.tile_from(id_d[:, :])
            recip = cpool.tile([128, NTIL], F32)
            neg1 = cpool.tile([128, 1], F32)
            nc.vector.memset(neg1[:], -1.0)
            pos1 = cpool.tile([128, 1], F32)
            nc.vector.memset(pos1[:], 1.0)
            hT_cache = cpool.tile([64, NTIL * 128], BF16)
            out_sb = cpool.tile([1, NLOC], F32)

            hw2l_loc = dp.tile([NLOC, 32], BF16)
            if AG:
                hw2l_all = dp.tile([NT, 32], BF16)
            else:
                hw2l_all = hw2l_loc
            l2tA = dp.tile([HALF, 128], BF16)
            l2tB = dp.tile([NT - HALF, 128], BF16)

            qctr = [0]
            TOTH = [int(sum(nch[t2, 0] for t2 in range(NTIL))),
                    int(sum(nch[t2, 1] for t2 in range(NTIL)))]
            stream = {0: [], 1: []}

            def ensure_call(h, ci_, tabA, tabB):
                lst = stream[h]
                base = 0 if h == 0 else TOTH[0]
                tabs = tabA if h == 0 else tabB
                while len(lst) <= ci_:
                    j = len(lst) * 8
                    k = min(8, TOTH[h] - j)
                    og = base + j
                    g = gp.tile([128, k, 128], BF16, tag="G")
                    nc.gpsimd.dma_gather(
                        g[:], tabs[:], idx_sb[:, og * 8:(og + k) * 8],
                        k * 128, k * 128, 128, queue_num=qctr[0] % 4)
                    qctr[0] += 1
                    ob = sb.tile([128, k, 128], BF16, tag="O")
                    nc.vector.tensor_tensor(
                        out=ob[:],
                        in0=ci[:, None, :].to_broadcast([128, k, 128]),
                        in1=dstl_sb[:, og:og + k, None].to_broadcast([128, k, 128]),
                        op=mybir.AluOpType.is_equal)
                    lst.append((g, ob, k))
                return lst[ci_]

            def agg_layer(t, tabA, tabB, width, ps_pool):
                """One-hot aggregation for tile t -> psum [128, width]."""
                ps = ps_pool.tile([128, width], F32, tag="agg")
                total = int(nch[t, 0] + nch[t, 1])
                done = 0
                for h in (0, 1):
                    off_t = ch_off[(t, h)] - (0 if h == 0 else TOTH[0])
                    for l in range(int(nch[t, h])):
                        sc = off_t + l
                        g, ob, k = ensure_call(h, sc // 8, tabA, tabB)
                        c = sc % 8
                        nc.tensor.matmul(
                            out=ps[:], lhsT=ob[:, c, :], rhs=g[:, c, :width],
                            start=(done == 0), stop=(done == total - 1))
                        done += 1
                return ps

            # ---------------- Layer 1 ----------------
            for t in range(NTIL):
                ps = agg_layer(t, xtabA, xtabB, 65, pa)
                tmp1 = sb.tile([128, 1], F32, tag="c1")
                nc.vector.tensor_scalar_max(tmp1[:], ps[:, 64:65], 1.0)
                nc.vector.reciprocal(recip[:, t:t + 1], tmp1[:])
                aggs = sb.tile([128, 64], BF16, tag="aggs")
                nc.vector.tensor_scalar(
                    out=aggs[:], in0=ps[:, :64], scalar1=recip[:, t:t + 1],
                    scalar2=None, op0=mybir.AluOpType.mult)
                pT = pb.tile([64, 128], BF16, tag="pb")
                nc.tensor.transpose(out=pT[:], in_=aggs[:], identity=ident[:])
                aggT = sb.tile([64, 128], BF16, tag="aggT")
                nc.any.tensor_copy(out=aggT[:], in_=pT[:])
                pH = pb.tile([128, 64], F32, tag="pb")
                nc.tensor.matmul(out=pH[:], lhsT=aggT[:], rhs=w1l[:],
                                 start=True, stop=False)
                nc.tensor.matmul(out=pH[:], lhsT=xT_sb[:, t * 128:(t + 1) * 128],
                                 rhs=w1r[:], start=False, stop=True)
                hf = sb.tile([128, 64], F32, tag="hf")
                nc.vector.tensor_tensor(out=hf[:], in0=pH[:], in1=b1[:],
                                        op=mybir.AluOpType.add)
                hb = sb.tile([128, 64], BF16, tag="hb")
                nc.scalar.activation(hb[:], hf[:], mybir.ActivationFunctionType.Relu)
                pT2 = pb.tile([64, 128], BF16, tag="pb")
                nc.tensor.transpose(out=pT2[:], in_=hb[:], identity=ident[:])
                hTs = hT_cache[:, t * 128:(t + 1) * 128]
                nc.any.tensor_copy(out=hTs, in_=pT2[:])
                pW = pb.tile([128, 32], F32, tag="pb")
                nc.tensor.matmul(out=pW[:], lhsT=hTs, rhs=w2l[:],
                                 start=True, stop=True)
                wsb = sb.tile([128, 32], BF16, tag="wsb")
                nc.any.tensor_copy(out=wsb[:], in_=pW[:])
                nc.sync.dma_start(out=hw2l_loc[t * 128:(t + 1) * 128, :], in_=wsb[:])

            # ---------------- AllGather + L2 table expand ----------------
            if AG:
                nc.gpsimd.collective_compute(
                    "AllGather", mybir.AluOpType.bypass,
                    replica_groups=[list(range(NC))],
                    ins=[hw2l_loc.opt()], outs=[hw2l_all.opt()])
            HH = HALF // 2
            nc.sync.dma_start(out=l2tA[0:HH, :32], in_=hw2l_all[0:HH, :])
            nc.scalar.dma_start(out=l2tA[HH:HALF, :32], in_=hw2l_all[HH:HALF, :])
            nc.sync.dma_start(out=l2tB[0:HH, :32], in_=hw2l_all[HALF:HALF + HH, :])
            nc.scalar.dma_start(out=l2tB[HH:NT - HALF, :32],
                                in_=hw2l_all[HALF + HH:NT, :])

            # ---------------- Layer 2 ----------------
            stream[0], stream[1] = [], []
            for t in range(NTIL):
                ps = agg_layer(t, l2tA, l2tB, 32, pa)
                a2 = sb.tile([128, 32], F32, tag="a2")
                nc.vector.tensor_scalar(
                    out=a2[:], in0=ps[:], scalar1=recip[:, t:t + 1],
                    scalar2=None, op0=mybir.AluOpType.mult)
                pH2 = pb.tile([128, 32], F32, tag="pb")
                nc.tensor.matmul(out=pH2[:], lhsT=hT_cache[:, t * 128:(t + 1) * 128],
                                 rhs=w2r[:], start=True, stop=True)
                h2f = sb.tile([128, 32], F32, tag="h2f")
                nc.vector.tensor_tensor(out=h2f[:], in0=pH2[:], in1=a2[:],
                                        op=mybir.AluOpType.add)
                nc.vector.tensor_tensor(out=h2f[:], in0=h2f[:], in1=b2[:],
                                        op=mybir.AluOpType.add)
                h2b = sb.tile([128, 32], BF16, tag="h2b")
                nc.scalar.activation(h2b[:], h2f[:], mybir.ActivationFunctionType.Relu)
                pT3 = pb.tile([32, 128], BF16, tag="pb")
                nc.tensor.transpose(out=pT3[:], in_=h2b[:], identity=ident[:])
                h2T = sb.tile([32, 128], BF16, tag="h2T")
                nc.any.tensor_copy(out=h2T[:], in_=pT3[:])
                pO = pb.tile([1, 128], F32, tag="pb")
                nc.tensor.matmul(out=pO[:], lhsT=wln[:], rhs=h2T[:],
                                 start=True, stop=True)
                nc.vector.tensor_scalar(
                    out=out_sb[0:1, t * 128:(t + 1) * 128], in0=pO[:],
                    scalar1=bl[0:1, 0:1], scalar2=None, op0=mybir.AluOpType.add)
                if t % 12 == 11 or t == NTIL - 1:
                    lo = (t // 12) * 12 * 128
                    nc.sync.dma_start(out=out_d[:, lo:(t + 1) * 128],
                                      in_=out_sb[0:1, lo:(t + 1) * 128])
    nc.compile()
    return nc


def _run(x, edge_index, W1_l, b1_l, W1_r, W2_l, b2_l, W2_r, W_lin, b_lin, cfg,
         trace=False):
    global _LAST_EXEC_NS
    N, NC, NPC, NLOC, NTIL, HALF, NT = (cfg["N"], cfg["NC"], cfg["NPC"],
                                        cfg["NLOC"], cfg["NTIL"], cfg["HALF"],
                                        cfg["NTAB"])
    x = np.asarray(x, dtype=np.float32)
    idx_arr, dst_arr, nch, ch_off, tot_ch = _prep_edges(edge_index, cfg)
    nc = _build(cfg, nch, ch_off, tot_ch)

    bf = ml_dtypes.bfloat16
    # gather table: rows [x | 1 | 0...] in permuted (padded) node order
    xtab = np.zeros((NT, 128), dtype=bf)
    xp = np.zeros((NT, 64), dtype=np.float32)
    for c in range(NC):
        xp[c * NLOC:c * NLOC + NPC] = x[c * NPC:(c + 1) * NPC]
    xtab[:, :64] = xp.astype(bf)
    xtab[:, 64] = bf(1.0)
    xtabA, xtabB = xtab[:HALF], xtab[HALF:]

    b1_bc = np.tile(np.asarray(b1_l, np.float32)[None, :], (128, 1))
    b2_bc = np.tile(np.asarray(b2_l, np.float32)[None, :], (128, 1))
    bl_bc = np.asarray(b_lin, np.float32).reshape(1, 1)
    ciota = np.tile(np.arange(128, dtype=np.float32)[None, :], (128, 1))
    ident = np.eye(128, dtype=np.float32).astype(bf)

    common = {
        "xtabA": xtabA, "xtabB": xtabB,
        "W1lT": np.asarray(W1_l, np.float32).T.copy().astype(bf),
        "W1rT": np.asarray(W1_r, np.float32).T.copy().astype(bf),
        "W2lT": np.asarray(W2_l, np.float32).T.copy().astype(bf),
        "W2rT": np.asarray(W2_r, np.float32).T.copy().astype(bf),
        "WlinT": np.asarray(W_lin, np.float32).T.copy().astype(bf),
        "b1": b1_bc, "b2": b2_bc, "blin": bl_bc,
        "Ciota": ciota, "Ident": ident,
    }
    in_maps = []
    for c in range(NC):
        xl = np.zeros((NLOC, 64), dtype=np.float32)
        xl[:NPC] = x[c * NPC:(c + 1) * NPC]
        m = dict(common)
        m["idx"] = idx_arr[c]
        m["dstl"] = np.asarray(dst_arr[c])
        m["xT"] = xl.T.copy().astype(bf)
        in_maps.append(m)

    res = run_bass_kernel_spmd(nc, in_maps, core_ids=list(range(NC)), trace=trace)
    _LAST_EXEC_NS = res.exec_time_ns
    out = np.zeros((N, 1), dtype=np.float32)
    for c in range(NC):
        out[c * NPC:(c + 1) * NPC, 0] = res.results[c]["out"][0, :NPC]
    return out


def _mkcfg(N, NC):
    NPC = N // NC
    NTIL = (NPC + 127) // 128
    NLOC = NTIL * 128
    NT = NC * NLOC
    return {"N": N, "NC": NC, "NPC": NPC, "NTIL": NTIL, "NLOC": NLOC,
            "NTAB": NT, "HALF": NT // 2}


def kernel(x, edge_index, W1_l, b1_l, W1_r, W2_l, b2_l, W2_r, W_lin, b_lin):
    cfg = _mkcfg(50000, 8)
    return _run(x, edge_index, W1_l, b1_l, W1_r, W2_l, b2_l, W2_r, W_lin, b_lin,
                cfg, trace=os.environ.get("BASS_GNN_TRACE", "0") == "1")
